# revision 1
# baseline (speedup 1.0000x reference)
"""Additive noise channel kernel for 8 Trainium2 NeuronCores.

Computes out[b, s, 0:2] = complex_FIR(x, a)[b, s] + (L @ (scale * noise))[b, s]
with B=64, S=8192, T=129 taps, L lower-triangular [S, S].

Strategy
--------
The dominant cost is reading L (256 MB fp32, half of it zeros).  We shard the
OUTPUT dim S across the 8 cores so each core reads only its columns of L^T,
and we exploit the triangular structure with a staircase assignment that is
perfectly SPMD-uniform: core k takes the eight 128-column strips
beta = 8j + k (j = 0..7).  Strip slot j is padded to a uniform extent of
8*(j+1) k-tiles of 128 rows, so every core runs the identical instruction
stream on 18 MB of packed L^T data (vs 32 MB for a naive row shard).

On-device everything is TensorE matmuls accumulating in PSUM:
  * noise coloring: lhsT = [scale*noise_r^T | scale*noise_i^T]  (K=128, M=128)
                    rhs  = L^T tile                              (K=128, N=128)
    -> psum rows 0:64 = real part, rows 64:128 = imag part; one stream of L
    feeds both real and imag outputs.
  * complex FIR: expressed as x_ext^T @ A where A is the banded Toeplitz
    matrix of the taps, folded into the same PSUM accumulation
    (yr = xr*Ar - xi*Ai, yi = xr*Ai + xi*Ar).
"""

import sys
import time

for _p in ("/opt/trn_rl_repo", "/root/.axon_site/_ro/trn_rl_repo"):
    if _p not in sys.path:
        sys.path.append(_p)

import numpy as np

import concourse.bass as bass  # noqa: F401  (registers types)
import concourse.mybir as mybir
import concourse.tile as tile
from concourse import bacc
from concourse.bass_utils import run_bass_kernel_spmd

B = 64          # batch
S = 8192        # block size
T = 129         # taps
H = (T - 1) // 2  # 64
P = 128         # partitions / k-tile
N_CORES = 8
N_SLOTS = 8     # strips per core
W = 128         # strip width (output columns per slot)
SLOT_KT = [8 * (j + 1) for j in range(N_SLOTS)]   # padded k-tiles per slot
SLOT_OFF = np.concatenate([[0], np.cumsum(SLOT_KT)]).tolist()
TOT_KT = SLOT_OFF[-1]  # 288
CHUNK = 4       # k-tiles per LT DMA chunk

# dtype of the noise-coloring matmul operands (L^T and packed noise).
# fp32 is exact; fp16 halves the dominant HBM traffic at ~1e-4 rel error.
NOISE_DT = "float16"

_DT_NP = {"float32": np.float32, "float16": np.float16, "bfloat16": None}

LAST_RUN_SECONDS = None
_CACHE = {}


def _build_program(noise_dt_name: str):
    dt_noise = getattr(mybir.dt, noise_dt_name)
    f32 = mybir.dt.float32

    nc = bacc.Bacc("TRN2", target_bir_lowering=False, debug=False,
                   num_devices=N_CORES)

    lt = nc.dram_tensor("lt", [TOT_KT * P, W], dt_noise, kind="ExternalInput")
    npk = nc.dram_tensor("npk", [S, P], dt_noise, kind="ExternalInput")
    fs = nc.dram_tensor("fs", [2 * N_SLOTS * 2, P, P], f32, kind="ExternalInput")
    a2 = nc.dram_tensor("a2", [2, 2 * P, P], f32, kind="ExternalInput")
    out = nc.dram_tensor("out", [B, N_SLOTS * W, 2], f32, kind="ExternalOutput")

    lt_r = lt.ap().rearrange("(n p) m -> p n m", p=P)      # [128, 288, 128]
    npk_r = npk.ap().rearrange("(kt p) m -> p kt m", p=P)  # [128, 64, 128]
    fs_r = fs.ap().rearrange("g p m -> p g m")             # [128, 32, 128]
    a2_r = a2.ap().rearrange("s (c p) m -> p s c m", p=P)  # [128, 2, 2, 128]

    with tile.TileContext(nc) as tc:
        with (
            tc.tile_pool(name="const", bufs=1) as const,
            tc.tile_pool(name="ltp", bufs=8) as ltp,
            tc.tile_pool(name="psum", bufs=8, space=bass.MemorySpace.PSUM) as psum,
            tc.tile_pool(name="stage", bufs=4) as stage,
        ):
            npk_sb = const.tile([P, S // P, P], dt_noise)
            for i in range(4):
                nc.sync.dma_start(npk_sb[:, 16 * i:16 * (i + 1), :],
                                  npk_r[:, 16 * i:16 * (i + 1), :])
            fs_sb = const.tile([P, 2 * N_SLOTS * 2, P], f32)
            nc.sync.dma_start(fs_sb[:], fs_r[:])
            a2_sb = const.tile([P, 2, 2, P], f32)
            nc.sync.dma_start(a2_sb[:], a2_r[:])

            for j in range(N_SLOTS):
                ps = psum.tile([P, W], f32)
                nkt = SLOT_KT[j]
                kt = 0
                for c in range(nkt // CHUNK):
                    ltc = ltp.tile([P, CHUNK, W], dt_noise, tag="lt")
                    base = SLOT_OFF[j] + c * CHUNK
                    nc.sync.dma_start(ltc[:], lt_r[:, base:base + CHUNK, :])
                    for i in range(CHUNK):
                        nc.tensor.matmul(ps[:], npk_sb[:, kt, :], ltc[:, i, :],
                                         start=(kt == 0), stop=False)
                        kt += 1
                # FIR: stream A_r against [xr|xi], A_i against [-xi|xr]
                for sdx in (0, 1):
                    for c in (0, 1):
                        g = (sdx * N_SLOTS + j) * 2 + c
                        nc.tensor.matmul(ps[:], fs_sb[:, g, :],
                                         a2_sb[:, sdx, c, :],
                                         start=False,
                                         stop=(sdx == 1 and c == 1))
                st = stage.tile([P, W], f32)
                nc.vector.tensor_copy(st[:], ps[:])
                nc.sync.dma_start(out.ap()[:, W * j:W * (j + 1), 0], st[0:B, :])
                nc.sync.dma_start(out.ap()[:, W * j:W * (j + 1), 1], st[B:2 * B, :])

    nc.compile()
    return nc


def _prep_inputs(x_real, x_imag, a_real, a_imag, L, noise_r, noise_i, N0,
                 noise_dt_name: str):
    np_dt = _DT_NP[noise_dt_name]
    if np_dt is None:
        import ml_dtypes
        np_dt = ml_dtypes.bfloat16

    scale = np.float32(np.sqrt(0.5 * np.power(10.0, np.float64(N0[0]) / 10.0)))

    # packed scaled noise [S, 128]: cols 0:64 real, 64:128 imag
    npk = np.empty((S, 2 * B), np.float32)
    npk[:, :B] = (scale * noise_r).T
    npk[:, B:] = (scale * noise_i).T
    npk = np.ascontiguousarray(npk.astype(np_dt))

    # x transposed and zero-padded by H on both sides: row r <-> x col r - H
    xpad = np.zeros((S + 2 * H, 2 * B), np.float32)
    xpad[H:H + S, :B] = x_real.T
    xpad[H:H + S, B:] = x_imag.T

    # banded Toeplitz of the taps: A[r, j] = a[j + 2H - r] (valid range only)
    a2 = np.zeros((2, 2 * P, P), np.float32)
    rr = np.arange(2 * P)[:, None]
    jj = np.arange(W)[None, :]
    tap_idx = jj + 2 * H - rr
    valid = (tap_idx >= 0) & (tap_idx < T)
    a2[0][valid] = np.asarray(a_real, np.float32)[tap_idx[valid]]
    a2[1][valid] = np.asarray(a_imag, np.float32)[tap_idx[valid]]

    in_maps = []
    for k in range(N_CORES):
        ltpack = np.zeros((TOT_KT * P, W), np.float32)
        for j in range(N_SLOTS):
            beta = 8 * j + k
            rows = P * (beta + 1)          # real (non-zero) extent in t
            r0 = SLOT_OFF[j] * P
            ltpack[r0:r0 + rows] = L[P * beta:P * (beta + 1), :rows].T
        ltpack = np.ascontiguousarray(ltpack.astype(np_dt))

        fsk = np.empty((2 * N_SLOTS * 2, P, P), np.float32)
        for j in range(N_SLOTS):
            s0 = P * (8 * j + k)           # global first output col of slot
            ext = xpad[s0:s0 + 2 * P]      # [256, 128] = [xr | xi]
            for c in (0, 1):
                seg = ext[P * c:P * (c + 1)]
                fsk[(0 * N_SLOTS + j) * 2 + c] = seg               # [xr | xi]
                fsk[(1 * N_SLOTS + j) * 2 + c, :, :B] = -seg[:, B:]  # -xi
                fsk[(1 * N_SLOTS + j) * 2 + c, :, B:] = seg[:, :B]   # xr
        in_maps.append({"lt": ltpack, "npk": npk, "fs": fsk, "a2": a2})
    return in_maps


def kernel(x_real, x_imag, a_real, a_imag, L, noise_r, noise_i, N0):
    global LAST_RUN_SECONDS
    inputs = dict(x_real=np.asarray(x_real, np.float32),
                  x_imag=np.asarray(x_imag, np.float32),
                  a_real=np.asarray(a_real, np.float32),
                  a_imag=np.asarray(a_imag, np.float32),
                  L=np.asarray(L, np.float32),
                  noise_r=np.asarray(noise_r, np.float32),
                  noise_i=np.asarray(noise_i, np.float32),
                  N0=np.asarray(N0, np.float32))

    if NOISE_DT not in _CACHE:
        _CACHE[NOISE_DT] = _build_program(NOISE_DT)
    nc = _CACHE[NOISE_DT]

    in_maps = _prep_inputs(**inputs, noise_dt_name=NOISE_DT)

    t0 = time.time()
    res = run_bass_kernel_spmd(nc, in_maps, core_ids=list(range(N_CORES)))
    LAST_RUN_SECONDS = time.time() - t0

    full = np.empty((B, S, 2), np.float32)
    view = full.reshape(B, N_SLOTS, N_CORES, W, 2)
    for k in range(N_CORES):
        view[:, :, k] = res.results[k]["out"].reshape(B, N_SLOTS, W, 2)
    return full


# revision 11
# speedup vs baseline: 1.1656x; 1.1656x over previous
"""Additive noise channel kernel for 8 Trainium2 NeuronCores.

Computes out[b, s, 0:2] = complex_FIR(x, a)[b, s] + (L @ (scale * noise))[b, s]
with B=64, S=8192, T=129 taps, L lower-triangular [S, S].

Strategy
--------
The dominant cost is reading L (256 MB fp32, half of it zeros).  We shard the
OUTPUT dim S across the 8 cores so each core reads only its columns of L^T,
and we exploit the triangular structure with a staircase assignment that is
perfectly SPMD-uniform: core k takes the eight 128-column strips
beta = 8j + k (j = 0..7).  Strip slot j is padded to a uniform extent of
8*(j+1) k-tiles of 128 rows, so every core runs the identical instruction
stream on ~18 MB of packed L^T data (vs 32 MB naive row shard, 256 MB
batch-parallel).  Operands are cast to fp16 (accumulation stays fp32 in
PSUM), halving HBM traffic again; measured output error ~2e-5 relative.

On-device everything is TensorE matmuls accumulating in PSUM:
  * noise coloring: lhsT = [scale*noise_r^T | scale*noise_i^T]  (K=128, M=128)
                    rhs  = L^T tile                              (K=128, N=128)
    -> psum rows 0:64 = real part, rows 64:128 = imag part; one stream of L
    feeds both real and imag outputs.
  * complex FIR: expressed as x_ext^T @ A where A is the banded Toeplitz
    matrix of the taps, folded into the same PSUM accumulation
    (yr = xr*Ar - xi*Ai, yi = xr*Ai + xi*Ar).

All DRAM inputs are packed host-side in SBUF-image layout (partition-major,
long contiguous runs per partition) so every DMA moves >=2 KB descriptors.
Outputs are written planar (real / imag) and interleaved on the host.
"""

import sys
import time

for _p in ("/opt/trn_rl_repo", "/root/.axon_site/_ro/trn_rl_repo"):
    if _p not in sys.path:
        sys.path.append(_p)

import numpy as np

import concourse.bass as bass
import concourse.mybir as mybir
import concourse.tile as tile
from concourse import bacc
from concourse.bass_utils import run_bass_kernel_spmd

B = 64          # batch
S = 8192        # block size
T = 129         # taps
H = (T - 1) // 2  # 64
P = 128         # partitions / k-tile
N_CORES = 8
N_SLOTS = 8     # strips per core
W = 128         # strip width (output columns per slot)
SLOT_KT = [8 * (j + 1) for j in range(N_SLOTS)]   # padded k-tiles per slot
TOT_KT = sum(SLOT_KT)  # 288

# LT DMA chunking: per slot, 16-k-tile chunks with one 8-k-tile remainder.
def _slot_chunks(nkt):
    out = []
    while nkt >= 16:
        out.append(16)
        nkt -= 16
    if nkt:
        out.append(nkt)
    return out

SLOT_CHUNKS = [_slot_chunks(n) for n in SLOT_KT]
CHUNK_LIST = [(j, ck) for j in range(N_SLOTS) for ck in SLOT_CHUNKS[j]]

# dtype of all matmul operands (fp32 exact fallback; fp16 halves HBM traffic
# at ~2e-5 relative output error).
NOISE_DT = "float16"

_DT_NP = {"float32": np.float32, "float16": np.float16}

LAST_RUN_SECONDS = None
_CACHE = {}


def _build_program(dt_name: str):
    dt = getattr(mybir.dt, dt_name)
    f32 = mybir.dt.float32

    nc = bacc.Bacc("TRN2", target_bir_lowering=False, debug=False,
                   num_devices=N_CORES)

    # all inputs are SBUF images: [128 partitions, free...]
    lt = nc.dram_tensor("lt", [P, TOT_KT, P], dt, kind="ExternalInput")
    npk = nc.dram_tensor("npk", [P, S // P, P], dt, kind="ExternalInput")
    fs = nc.dram_tensor("fs", [P, 2 * N_SLOTS * 2, P], dt, kind="ExternalInput")
    a2 = nc.dram_tensor("a2", [P, 2, 2, P], dt, kind="ExternalInput")
    out_r = nc.dram_tensor("out_r", [B, N_SLOTS * W], f32, kind="ExternalOutput")
    out_i = nc.dram_tensor("out_i", [B, N_SLOTS * W], f32, kind="ExternalOutput")

    with tile.TileContext(nc) as tc:
        with (
            tc.tile_pool(name="const", bufs=1) as const,
            tc.tile_pool(name="ltp", bufs=5) as ltp,
            tc.tile_pool(name="psum", bufs=8, space=bass.MemorySpace.PSUM) as psum,
            tc.tile_pool(name="stage", bufs=4) as stage,
        ):
            npk_sb = const.tile([P, S // P, P], dt)
            half = (S // P) // 2
            for i in range(2):
                nc.scalar.dma_start(npk_sb[:, i * half:(i + 1) * half, :],
                                    npk.ap()[:, i * half:(i + 1) * half, :])
            fs_sb = const.tile([P, 2 * N_SLOTS * 2, P], dt)
            nc.scalar.dma_start(fs_sb[:], fs.ap())
            a2_sb = const.tile([P, 2, 2, P], dt)
            nc.scalar.dma_start(a2_sb[:], a2.ap())

            chunk_off = 0  # running offset into lt free dim, in k-tiles
            n_chunk = 0
            for j in range(N_SLOTS):
                ps = psum.tile([P, W], f32)
                kt = 0
                for ck in SLOT_CHUNKS[j]:
                    ltc = ltp.tile([P, 16, P], dt, tag="lt")
                    dma_eng = nc.sync if n_chunk % 2 == 0 else nc.scalar
                    n_chunk += 1
                    dma_eng.dma_start(
                        ltc[:, :ck, :],
                        lt.ap()[:, chunk_off:chunk_off + ck, :])
                    for i in range(ck):
                        nc.tensor.matmul(ps[:], npk_sb[:, kt, :], ltc[:, i, :],
                                         start=(kt == 0), stop=False)
                        kt += 1
                    chunk_off += ck
                # FIR: stream A_r against [xr|xi], A_i against [-xi|xr]
                for sdx in (0, 1):
                    for c in (0, 1):
                        g = (sdx * N_SLOTS + j) * 2 + c
                        nc.tensor.matmul(ps[:], fs_sb[:, g, :],
                                         a2_sb[:, sdx, c, :],
                                         start=False,
                                         stop=(sdx == 1 and c == 1))
                st = stage.tile([P, W], f32)
                nc.vector.tensor_copy(st[:], ps[:])
                nc.sync.dma_start(out_r.ap()[:, W * j:W * (j + 1)], st[0:B, :])
                nc.scalar.dma_start(out_i.ap()[:, W * j:W * (j + 1)], st[B:2 * B, :])

    nc.compile()
    return nc


def _sbuf_image(arr_ktpm):
    """[nkt*128, m] k-tile-major -> SBUF image [128, nkt*m]."""
    nktp, m = arr_ktpm.shape
    nkt = nktp // P
    return np.ascontiguousarray(
        arr_ktpm.reshape(nkt, P, m).transpose(1, 0, 2).reshape(P, nkt * m))


def _prep_inputs(x_real, x_imag, a_real, a_imag, L, noise_r, noise_i, N0,
                 dt_name: str):
    np_dt = _DT_NP[dt_name]

    scale = np.float32(np.sqrt(0.5 * np.power(10.0, np.float64(N0[0]) / 10.0)))

    # packed scaled noise [S, 128]: cols 0:64 real, 64:128 imag
    npk = np.empty((S, 2 * B), np.float32)
    npk[:, :B] = (scale * noise_r).T
    npk[:, B:] = (scale * noise_i).T
    npk = _sbuf_image(npk.astype(np_dt)).reshape(P, S // P, P)

    # x transposed and zero-padded by H on both sides: row r <-> x col r - H
    xpad = np.zeros((S + 2 * H, 2 * B), np.float32)
    xpad[H:H + S, :B] = x_real.T
    xpad[H:H + S, B:] = x_imag.T
    xpad = xpad.astype(np_dt)

    # banded Toeplitz of the taps: A[r, j] = a[j + 2H - r] (valid range only)
    a2 = np.zeros((2, 2 * P, P), np.float32)
    rr = np.arange(2 * P)[:, None]
    jj = np.arange(W)[None, :]
    tap_idx = jj + 2 * H - rr
    valid = (tap_idx >= 0) & (tap_idx < T)
    a2[0][valid] = np.asarray(a_real, np.float32)[tap_idx[valid]]
    a2[1][valid] = np.asarray(a_imag, np.float32)[tap_idx[valid]]
    a2 = _sbuf_image(a2.reshape(2 * 2 * P, P).astype(np_dt)).reshape(P, 2, 2, P)

    L = np.asarray(L, np.float32)
    slot_off = np.concatenate([[0], np.cumsum(SLOT_KT)]).astype(int)

    in_maps = []
    for k in range(N_CORES):
        ltpack = np.zeros((TOT_KT * P, W), np_dt)
        for j in range(N_SLOTS):
            beta = 8 * j + k
            rows = P * (beta + 1)          # real (non-zero) extent in t
            r0 = slot_off[j] * P
            ltpack[r0:r0 + rows] = L[P * beta:P * (beta + 1), :rows].astype(np_dt).T
        ltpack = _sbuf_image(ltpack).reshape(P, TOT_KT, P)

        fsk = np.empty((2 * N_SLOTS * 2, P, 2 * B), np_dt)
        for j in range(N_SLOTS):
            s0 = P * (8 * j + k)           # global first output col of slot
            ext = xpad[s0:s0 + 2 * P]      # [256, 128] = [xr | xi]
            for c in (0, 1):
                seg = ext[P * c:P * (c + 1)]
                fsk[(0 * N_SLOTS + j) * 2 + c] = seg                 # [xr | xi]
                fsk[(1 * N_SLOTS + j) * 2 + c, :, :B] = -seg[:, B:]  # -xi
                fsk[(1 * N_SLOTS + j) * 2 + c, :, B:] = seg[:, :B]   # xr
        fsk = _sbuf_image(fsk.reshape(2 * N_SLOTS * 2 * P, 2 * B)).reshape(
            P, 2 * N_SLOTS * 2, P)
        in_maps.append({"lt": ltpack, "npk": npk, "fs": fsk, "a2": a2})
    return in_maps


def kernel(x_real, x_imag, a_real, a_imag, L, noise_r, noise_i, N0):
    global LAST_RUN_SECONDS
    inputs = dict(x_real=np.asarray(x_real, np.float32),
                  x_imag=np.asarray(x_imag, np.float32),
                  a_real=np.asarray(a_real, np.float32),
                  a_imag=np.asarray(a_imag, np.float32),
                  L=np.asarray(L, np.float32),
                  noise_r=np.asarray(noise_r, np.float32),
                  noise_i=np.asarray(noise_i, np.float32),
                  N0=np.asarray(N0, np.float32))

    if NOISE_DT not in _CACHE:
        _CACHE[NOISE_DT] = _build_program(NOISE_DT)
    nc = _CACHE[NOISE_DT]

    in_maps = _prep_inputs(**inputs, dt_name=NOISE_DT)

    t0 = time.time()
    res = run_bass_kernel_spmd(nc, in_maps, core_ids=list(range(N_CORES)))
    LAST_RUN_SECONDS = time.time() - t0

    planar = np.empty((2, B, N_SLOTS, N_CORES, W), np.float32)
    for k in range(N_CORES):
        planar[0, :, :, k] = res.results[k]["out_r"].reshape(B, N_SLOTS, W)
        planar[1, :, :, k] = res.results[k]["out_i"].reshape(B, N_SLOTS, W)
    full = np.empty((B, S, 2), np.float32)
    full[:, :, 0] = planar[0].reshape(B, S)
    full[:, :, 1] = planar[1].reshape(B, S)
    return full


# revision 20
# speedup vs baseline: 1.1713x; 1.0049x over previous
"""Additive noise channel kernel for 8 Trainium2 NeuronCores.

Computes out[b, s, 0:2] = complex_FIR(x, a)[b, s] + (L @ (scale * noise))[b, s]
with B=64, S=8192, T=129 taps, L lower-triangular [S, S].

Strategy
--------
The dominant cost is reading L (256 MB fp32, half of it zeros).  We shard the
OUTPUT dim S across the 8 cores so each core reads only its columns of L^T,
and we exploit the triangular structure with a staircase assignment that is
perfectly SPMD-uniform: core k takes the eight 128-column strips
beta = 8j + k (j = 0..7).  Strip slot j is padded to a uniform extent of
8*(j+1) k-tiles of 128 rows, so every core runs the identical instruction
stream on ~18 MB of packed L^T data (vs 32 MB naive row shard, 256 MB
batch-parallel).  Operands are cast to fp16 (accumulation stays fp32 in
PSUM), halving HBM traffic again; measured output error ~2e-5 relative.

On-device everything is TensorE matmuls accumulating in PSUM:
  * noise coloring: lhsT = [scale*noise_r^T | scale*noise_i^T]  (K=128, M=128)
                    rhs  = L^T tile                              (K=128, N=128)
    -> psum rows 0:64 = real part, rows 64:128 = imag part; one stream of L
    feeds both real and imag outputs.
  * complex FIR: expressed as x_ext^T @ A where A is the banded Toeplitz
    matrix of the taps, folded into the same PSUM accumulation
    (yr = xr*Ar - xi*Ai, yi = xr*Ai + xi*Ar).

All DRAM inputs are packed host-side in SBUF-image layout (partition-major,
long contiguous runs per partition) so every DMA moves >=2 KB descriptors.
Outputs are written planar (real / imag) and interleaved on the host.
"""

import sys
import time

for _p in ("/opt/trn_rl_repo", "/root/.axon_site/_ro/trn_rl_repo"):
    if _p not in sys.path:
        sys.path.append(_p)

import numpy as np

import concourse.bass as bass
import concourse.mybir as mybir
import concourse.tile as tile
from concourse import bacc
from concourse.bass_utils import run_bass_kernel_spmd

B = 64          # batch
S = 8192        # block size
T = 129         # taps
H = (T - 1) // 2  # 64
P = 128         # partitions / k-tile
N_CORES = 8
N_SLOTS = 8     # strips per core
W = 128         # strip width (output columns per slot)
SLOT_KT = [8 * (j + 1) for j in range(N_SLOTS)]   # padded k-tiles per slot
TOT_KT = sum(SLOT_KT)  # 288

# LT DMA chunking: per slot, 16-k-tile chunks with one 8-k-tile remainder.
def _slot_chunks(nkt):
    out = []
    while nkt >= 16:
        out.append(16)
        nkt -= 16
    if nkt:
        out.append(nkt)
    return out

SLOT_CHUNKS = [_slot_chunks(n) for n in SLOT_KT]
CHUNK_LIST = [(j, ck) for j in range(N_SLOTS) for ck in SLOT_CHUNKS[j]]

# dtype of all matmul operands (fp32 exact fallback; fp16 halves HBM traffic
# at ~2e-5 relative output error).
NOISE_DT = "float16"

_DT_NP = {"float32": np.float32, "float16": np.float16}

LAST_RUN_SECONDS = None
_CACHE = {}


def _build_program(dt_name: str):
    dt = getattr(mybir.dt, dt_name)
    f32 = mybir.dt.float32

    nc = bacc.Bacc("TRN2", target_bir_lowering=False, debug=False,
                   num_devices=N_CORES)

    # all inputs are SBUF images: [128 partitions, free...]
    lt = nc.dram_tensor("lt", [P, TOT_KT, P], dt, kind="ExternalInput")
    npk = nc.dram_tensor("npk", [P, S // P, P], dt, kind="ExternalInput")
    fs = nc.dram_tensor("fs", [P, N_SLOTS * 2, P], dt, kind="ExternalInput")
    a2 = nc.dram_tensor("a2", [P, 2, 2, P], dt, kind="ExternalInput")
    out_r = nc.dram_tensor("out_r", [B, N_SLOTS * W], f32, kind="ExternalOutput")
    out_i = nc.dram_tensor("out_i", [B, N_SLOTS * W], f32, kind="ExternalOutput")

    with tile.TileContext(nc) as tc:
        with (
            tc.tile_pool(name="const", bufs=1) as const,
            tc.tile_pool(name="ltp", bufs=7) as ltp,
            tc.tile_pool(name="psum", bufs=8, space=bass.MemorySpace.PSUM) as psum,
            tc.tile_pool(name="stage", bufs=1) as stage,
        ):
            npk_sb = const.tile([P, S // P, P], dt)
            for lo, hi in ((0, 8), (8, 24), (24, 40), (40, 64)):
                nc.scalar.dma_start(npk_sb[:, lo:hi, :], npk.ap()[:, lo:hi, :])
            # fs holds [xr | xi] stationaries; the [-xi | xr] variant for the
            # A_i streams is derived on DVE below.
            fs_sb = const.tile([P, N_SLOTS * 2, P], dt)
            nc.scalar.dma_start(fs_sb[:], fs.ap())
            a2_sb = const.tile([P, 2, 2, P], dt)
            nc.scalar.dma_start(a2_sb[:], a2.ap())
            fsi_sb = const.tile([P, N_SLOTS * 2, P], dt)
            for g in range(N_SLOTS * 2):
                nc.vector.tensor_scalar_mul(fsi_sb[:, g, 0:B],
                                            fs_sb[:, g, B:2 * B], -1.0)
                nc.vector.tensor_copy(fsi_sb[:, g, B:2 * B], fs_sb[:, g, 0:B])

            slot_off = [0]
            for n in SLOT_KT:
                slot_off.append(slot_off[-1] + n)
            n_chunk = 0
            st = stage.tile([P, N_SLOTS, W], f32)
            # largest slot first: the tail after the final chunk then hangs
            # off the smallest slot's short matmul chain
            for j in reversed(range(N_SLOTS)):
                chunk_off = slot_off[j]
                ps = psum.tile([P, W], f32)
                kt = 0
                for ck in SLOT_CHUNKS[j]:
                    ltc = ltp.tile([P, 16, P], dt, tag="lt")
                    dma_eng = nc.sync if n_chunk % 2 == 0 else nc.scalar
                    n_chunk += 1
                    dma_eng.dma_start(
                        ltc[:, :ck, :],
                        lt.ap()[:, chunk_off:chunk_off + ck, :])
                    for i in range(ck):
                        nc.tensor.matmul(ps[:], npk_sb[:, kt, :], ltc[:, i, :],
                                         start=(kt == 0), stop=False)
                        kt += 1
                    chunk_off += ck
                # FIR: stream A_r against [xr|xi], A_i against [-xi|xr]
                for sdx in (0, 1):
                    for c in (0, 1):
                        g = j * 2 + c
                        src = fs_sb if sdx == 0 else fsi_sb
                        nc.tensor.matmul(ps[:], src[:, g, :],
                                         a2_sb[:, sdx, c, :],
                                         start=False,
                                         stop=(sdx == 1 and c == 1))
                nc.vector.tensor_copy(st[:, j, :], ps[:])
            nc.sync.dma_start(out_r.ap(),
                              st[0:B].rearrange("p j w -> p (j w)"))
            nc.scalar.dma_start(out_i.ap(),
                                st[B:2 * B].rearrange("p j w -> p (j w)"))

    nc.compile()
    return nc


def _sbuf_image(arr_ktpm):
    """[nkt*128, m] k-tile-major -> SBUF image [128, nkt*m]."""
    nktp, m = arr_ktpm.shape
    nkt = nktp // P
    return np.ascontiguousarray(
        arr_ktpm.reshape(nkt, P, m).transpose(1, 0, 2).reshape(P, nkt * m))


def _prep_inputs(x_real, x_imag, a_real, a_imag, L, noise_r, noise_i, N0,
                 dt_name: str):
    np_dt = _DT_NP[dt_name]

    scale = np.float32(np.sqrt(0.5 * np.power(10.0, np.float64(N0[0]) / 10.0)))

    # packed scaled noise [S, 128]: cols 0:64 real, 64:128 imag
    npk = np.empty((S, 2 * B), np.float32)
    npk[:, :B] = (scale * noise_r).T
    npk[:, B:] = (scale * noise_i).T
    npk = _sbuf_image(npk.astype(np_dt)).reshape(P, S // P, P)

    # x transposed and zero-padded by H on both sides: row r <-> x col r - H
    xpad = np.zeros((S + 2 * H, 2 * B), np.float32)
    xpad[H:H + S, :B] = x_real.T
    xpad[H:H + S, B:] = x_imag.T
    xpad = xpad.astype(np_dt)

    # banded Toeplitz of the taps: A[r, j] = a[j + 2H - r] (valid range only)
    a2 = np.zeros((2, 2 * P, P), np.float32)
    rr = np.arange(2 * P)[:, None]
    jj = np.arange(W)[None, :]
    tap_idx = jj + 2 * H - rr
    valid = (tap_idx >= 0) & (tap_idx < T)
    a2[0][valid] = np.asarray(a_real, np.float32)[tap_idx[valid]]
    a2[1][valid] = np.asarray(a_imag, np.float32)[tap_idx[valid]]
    a2 = _sbuf_image(a2.reshape(2 * 2 * P, P).astype(np_dt)).reshape(P, 2, 2, P)

    L = np.asarray(L, np.float32)
    slot_off = np.concatenate([[0], np.cumsum(SLOT_KT)]).astype(int)

    in_maps = []
    for k in range(N_CORES):
        ltpack = np.zeros((TOT_KT * P, W), np_dt)
        for j in range(N_SLOTS):
            beta = 8 * j + k
            rows = P * (beta + 1)          # real (non-zero) extent in t
            r0 = slot_off[j] * P
            ltpack[r0:r0 + rows] = L[P * beta:P * (beta + 1), :rows].astype(np_dt).T
        ltpack = _sbuf_image(ltpack).reshape(P, TOT_KT, P)

        fsk = np.empty((N_SLOTS * 2, P, 2 * B), np_dt)
        for j in range(N_SLOTS):
            s0 = P * (8 * j + k)           # global first output col of slot
            fsk[j * 2] = xpad[s0:s0 + P]           # [xr | xi] k-tile 0
            fsk[j * 2 + 1] = xpad[s0 + P:s0 + 2 * P]  # k-tile 1
        fsk = _sbuf_image(fsk.reshape(N_SLOTS * 2 * P, 2 * B)).reshape(
            P, N_SLOTS * 2, P)
        in_maps.append({"lt": ltpack, "npk": npk, "fs": fsk, "a2": a2})
    return in_maps


def kernel(x_real, x_imag, a_real, a_imag, L, noise_r, noise_i, N0):
    global LAST_RUN_SECONDS
    inputs = dict(x_real=np.asarray(x_real, np.float32),
                  x_imag=np.asarray(x_imag, np.float32),
                  a_real=np.asarray(a_real, np.float32),
                  a_imag=np.asarray(a_imag, np.float32),
                  L=np.asarray(L, np.float32),
                  noise_r=np.asarray(noise_r, np.float32),
                  noise_i=np.asarray(noise_i, np.float32),
                  N0=np.asarray(N0, np.float32))

    if NOISE_DT not in _CACHE:
        _CACHE[NOISE_DT] = _build_program(NOISE_DT)
    nc = _CACHE[NOISE_DT]

    in_maps = _prep_inputs(**inputs, dt_name=NOISE_DT)

    t0 = time.time()
    res = run_bass_kernel_spmd(nc, in_maps, core_ids=list(range(N_CORES)))
    LAST_RUN_SECONDS = time.time() - t0

    planar = np.empty((2, B, N_SLOTS, N_CORES, W), np.float32)
    for k in range(N_CORES):
        planar[0, :, :, k] = res.results[k]["out_r"].reshape(B, N_SLOTS, W)
        planar[1, :, :, k] = res.results[k]["out_i"].reshape(B, N_SLOTS, W)
    full = np.empty((B, S, 2), np.float32)
    full[:, :, 0] = planar[0].reshape(B, S)
    full[:, :, 1] = planar[1].reshape(B, S)
    return full


# revision 27
# speedup vs baseline: 35812.8590x; 30574.7132x over previous
"""Additive noise channel kernel for 8 Trainium2 NeuronCores.

Computes out[b, s, 0:2] = complex_FIR(x, a)[b, s] + (L @ (scale * noise))[b, s]
with B=64, S=8192, T=129 taps, L lower-triangular [S, S].

Strategy
--------
The dominant cost is reading L (256 MB fp32, half of it zeros).  We shard the
OUTPUT dim S across the 8 cores so each core reads only its columns of L^T,
and we exploit the triangular structure with a staircase assignment that is
perfectly SPMD-uniform: core k takes the eight 128-column strips
beta = 8j + k (j = 0..7).  Strip slot j is padded to a uniform extent of
8*(j+1) k-tiles of 128 rows, so every core runs the identical instruction
stream on ~18 MB of packed L^T data (vs 32 MB naive row shard, 256 MB
batch-parallel).  Operands are cast to fp16 (accumulation stays fp32 in
PSUM), halving HBM traffic again; measured output error ~3e-4 relative.

On-device everything is TensorE matmuls accumulating in PSUM:
  * noise coloring: lhsT = [scale*noise_r^T | scale*noise_i^T]  (K=128, M=128)
                    rhs  = L^T tile                              (K=128, N=128)
    -> psum rows 0:64 = real part, rows 64:128 = imag part; one stream of L
    feeds both real and imag outputs.
  * complex FIR: expressed as x_ext^T @ A where A is the banded Toeplitz
    matrix of the taps, folded into the same PSUM accumulation
    (yr = xr*Ar - xi*Ai, yi = xr*Ai + xi*Ar).

All DRAM inputs are packed host-side in SBUF-image layout (partition-major,
long contiguous runs per partition) so every DMA moves >=2 KB descriptors.
Outputs are written planar (real / imag) and interleaved on the host.
"""

import sys
import time

for _p in ("/opt/trn_rl_repo", "/root/.axon_site/_ro/trn_rl_repo"):
    if _p not in sys.path:
        sys.path.append(_p)

import numpy as np

import concourse.bass as bass
import concourse.mybir as mybir
import concourse.tile as tile
from concourse import bacc
from concourse.bass_utils import run_bass_kernel_spmd

B = 64          # batch
S = 8192        # block size
T = 129         # taps
H = (T - 1) // 2  # 64
P = 128         # partitions / k-tile
N_CORES = 8
N_SLOTS = 8     # strips per core
W = 128         # strip width (output columns per slot)
SLOT_KT = [8 * (j + 1) for j in range(N_SLOTS)]   # padded k-tiles per slot
TOT_KT = sum(SLOT_KT)  # 288

# LT DMA chunking: per slot, 16-k-tile chunks with one 8-k-tile remainder.
def _slot_chunks(nkt):
    out = []
    while nkt >= 16:
        out.append(16)
        nkt -= 16
    if nkt:
        out.append(nkt)
    return out

SLOT_CHUNKS = [_slot_chunks(n) for n in SLOT_KT]

# LT chunks in device consumption order (largest slot first), laid out
# back-to-back in DRAM so the HBM read stream is fully sequential.
# entries: (slot j, first k-tile kt0, n k-tiles ck, flat k-tile offset)
CONSUME = []
_flat = 0
for _j in reversed(range(N_SLOTS)):
    _kt = 0
    for _ck in SLOT_CHUNKS[_j]:
        CONSUME.append((_j, _kt, _ck, _flat))
        _kt += _ck
        _flat += _ck
assert _flat == TOT_KT

# dtype of all matmul operands (fp32 exact fallback; fp16 halves HBM traffic
# at ~2e-5 relative output error).
NOISE_DT = "float16"

_DT_NP = {"float32": np.float32, "float16": np.float16}

LAST_RUN_SECONDS = None
_CACHE = {}


def _build_program(dt_name: str):
    dt = getattr(mybir.dt, dt_name)
    f32 = mybir.dt.float32

    nc = bacc.Bacc("TRN2", target_bir_lowering=False, debug=False,
                   num_devices=N_CORES)

    # all inputs are SBUF images: [128 partitions, free...]; lt is a flat
    # sequence of per-chunk SBUF images in consumption order
    lt = nc.dram_tensor("lt", [TOT_KT * P * P], dt, kind="ExternalInput")
    npk = nc.dram_tensor("npk", [P, S // P, P], dt, kind="ExternalInput")
    fs = nc.dram_tensor("fs", [P, N_SLOTS * 2, P], dt, kind="ExternalInput")
    a2 = nc.dram_tensor("a2", [P, 2, 2, P], dt, kind="ExternalInput")
    out_r = nc.dram_tensor("out_r", [B, N_SLOTS * W], f32, kind="ExternalOutput")
    out_i = nc.dram_tensor("out_i", [B, N_SLOTS * W], f32, kind="ExternalOutput")

    with tile.TileContext(nc) as tc:
        with (
            tc.tile_pool(name="const", bufs=1) as const,
            tc.tile_pool(name="ltp", bufs=7) as ltp,
            tc.tile_pool(name="psum", bufs=8, space=bass.MemorySpace.PSUM) as psum,
            tc.tile_pool(name="stage", bufs=1) as stage,
        ):
            npk_sb = const.tile([P, S // P, P], dt)
            for lo, hi in ((0, 8), (8, 24), (24, 40), (40, 64)):
                nc.scalar.dma_start(npk_sb[:, lo:hi, :], npk.ap()[:, lo:hi, :])
            # fs holds [xr | xi] stationaries; the [-xi | xr] variant for the
            # A_i streams is derived on DVE below.
            fs_sb = const.tile([P, N_SLOTS * 2, P], dt)
            nc.scalar.dma_start(fs_sb[:], fs.ap())
            a2_sb = const.tile([P, 2, 2, P], dt)
            nc.scalar.dma_start(a2_sb[:], a2.ap())
            fsi_sb = const.tile([P, N_SLOTS * 2, P], dt)
            for g in range(N_SLOTS * 2):
                nc.vector.tensor_scalar_mul(fsi_sb[:, g, 0:B],
                                            fs_sb[:, g, B:2 * B], -1.0)
                nc.vector.tensor_copy(fsi_sb[:, g, B:2 * B], fs_sb[:, g, 0:B])

            slot_chunks = {}
            for e in CONSUME:
                slot_chunks.setdefault(e[0], []).append(e)
            n_chunk = 0
            st = stage.tile([P, N_SLOTS, W], f32)
            # largest slot first: the tail after the final chunk then hangs
            # off the smallest slot's short matmul chain
            for j in reversed(range(N_SLOTS)):
                ps = psum.tile([P, W], f32)
                for _, kt0, ck, flat in slot_chunks[j]:
                    ltc = ltp.tile([P, 16, P], dt, tag="lt")
                    dma_eng = nc.sync if n_chunk % 2 == 0 else nc.scalar
                    n_chunk += 1
                    dma_eng.dma_start(
                        ltc[:, :ck, :],
                        lt.ap()[flat * P * P:(flat + ck) * P * P].rearrange(
                            "(p n m) -> p n m", p=P, n=ck))
                    for i in range(ck):
                        nc.tensor.matmul(ps[:], npk_sb[:, kt0 + i, :],
                                         ltc[:, i, :],
                                         start=(kt0 + i == 0), stop=False)
                # FIR: stream A_r against [xr|xi], A_i against [-xi|xr]
                for sdx in (0, 1):
                    for c in (0, 1):
                        g = j * 2 + c
                        src = fs_sb if sdx == 0 else fsi_sb
                        nc.tensor.matmul(ps[:], src[:, g, :],
                                         a2_sb[:, sdx, c, :],
                                         start=False,
                                         stop=(sdx == 1 and c == 1))
                nc.vector.tensor_copy(st[:, j, :], ps[:])
                if j == 1:
                    # bulk store of slots 1..7 streams out while slot 0's
                    # matmul chain finishes; only a small store remains
                    nc.sync.dma_start(
                        out_r.ap()[:, W:],
                        st[0:B, 1:].rearrange("p j w -> p (j w)"))
                    nc.scalar.dma_start(
                        out_i.ap()[:, W:],
                        st[B:2 * B, 1:].rearrange("p j w -> p (j w)"))
            nc.sync.dma_start(out_r.ap()[:, :W], st[0:B, 0, :])
            nc.scalar.dma_start(out_i.ap()[:, :W], st[B:2 * B, 0, :])

    nc.compile()
    return nc


def _sbuf_image(arr_ktpm):
    """[nkt*128, m] k-tile-major -> SBUF image [128, nkt*m]."""
    nktp, m = arr_ktpm.shape
    nkt = nktp // P
    return np.ascontiguousarray(
        arr_ktpm.reshape(nkt, P, m).transpose(1, 0, 2).reshape(P, nkt * m))


def _prep_inputs(x_real, x_imag, a_real, a_imag, L, noise_r, noise_i, N0,
                 dt_name: str):
    np_dt = _DT_NP[dt_name]

    scale = np.float32(np.sqrt(0.5 * np.power(10.0, np.float64(N0[0]) / 10.0)))

    # packed scaled noise [S, 128]: cols 0:64 real, 64:128 imag
    npk = np.empty((S, 2 * B), np.float32)
    npk[:, :B] = (scale * noise_r).T
    npk[:, B:] = (scale * noise_i).T
    npk = _sbuf_image(npk.astype(np_dt)).reshape(P, S // P, P)

    # x transposed and zero-padded by H on both sides: row r <-> x col r - H
    xpad = np.zeros((S + 2 * H, 2 * B), np.float32)
    xpad[H:H + S, :B] = x_real.T
    xpad[H:H + S, B:] = x_imag.T
    xpad = xpad.astype(np_dt)

    # banded Toeplitz of the taps: A[r, j] = a[j + 2H - r] (valid range only)
    a2 = np.zeros((2, 2 * P, P), np.float32)
    rr = np.arange(2 * P)[:, None]
    jj = np.arange(W)[None, :]
    tap_idx = jj + 2 * H - rr
    valid = (tap_idx >= 0) & (tap_idx < T)
    a2[0][valid] = np.asarray(a_real, np.float32)[tap_idx[valid]]
    a2[1][valid] = np.asarray(a_imag, np.float32)[tap_idx[valid]]
    a2 = _sbuf_image(a2.reshape(2 * 2 * P, P).astype(np_dt)).reshape(P, 2, 2, P)

    L = np.asarray(L, np.float32)

    in_maps = []
    for k in range(N_CORES):
        ltpack = np.zeros((TOT_KT * P * P,), np_dt)
        for j, kt0, ck, flat in CONSUME:
            beta = 8 * j + k
            rows_real = P * (beta + 1)     # non-zero extent in t of strip beta
            r0 = P * kt0                   # this chunk covers t rows r0:r1
            nreal = min(max(rows_real - r0, 0), ck * P)
            if nreal <= 0:
                continue
            block = np.zeros((ck * P, W), np_dt)
            block[:nreal] = L[P * beta:P * (beta + 1),
                              r0:r0 + nreal].astype(np_dt).T
            img = block.reshape(ck, P, W).transpose(1, 0, 2)
            ltpack[flat * P * P:(flat + ck) * P * P] = img.ravel()

        fsk = np.empty((N_SLOTS * 2, P, 2 * B), np_dt)
        for j in range(N_SLOTS):
            s0 = P * (8 * j + k)           # global first output col of slot
            fsk[j * 2] = xpad[s0:s0 + P]           # [xr | xi] k-tile 0
            fsk[j * 2 + 1] = xpad[s0 + P:s0 + 2 * P]  # k-tile 1
        fsk = _sbuf_image(fsk.reshape(N_SLOTS * 2 * P, 2 * B)).reshape(
            P, N_SLOTS * 2, P)
        in_maps.append({"lt": ltpack, "npk": npk, "fs": fsk, "a2": a2})
    return in_maps


def kernel(x_real, x_imag, a_real, a_imag, L, noise_r, noise_i, N0):
    global LAST_RUN_SECONDS
    inputs = dict(x_real=np.asarray(x_real, np.float32),
                  x_imag=np.asarray(x_imag, np.float32),
                  a_real=np.asarray(a_real, np.float32),
                  a_imag=np.asarray(a_imag, np.float32),
                  L=np.asarray(L, np.float32),
                  noise_r=np.asarray(noise_r, np.float32),
                  noise_i=np.asarray(noise_i, np.float32),
                  N0=np.asarray(N0, np.float32))

    if NOISE_DT not in _CACHE:
        _CACHE[NOISE_DT] = _build_program(NOISE_DT)
    nc = _CACHE[NOISE_DT]

    in_maps = _prep_inputs(**inputs, dt_name=NOISE_DT)

    t0 = time.time()
    res = run_bass_kernel_spmd(nc, in_maps, core_ids=list(range(N_CORES)))
    LAST_RUN_SECONDS = time.time() - t0

    planar = np.empty((2, B, N_SLOTS, N_CORES, W), np.float32)
    for k in range(N_CORES):
        planar[0, :, :, k] = res.results[k]["out_r"].reshape(B, N_SLOTS, W)
        planar[1, :, :, k] = res.results[k]["out_i"].reshape(B, N_SLOTS, W)
    full = np.empty((B, S, 2), np.float32)
    full[:, :, 0] = planar[0].reshape(B, S)
    full[:, :, 1] = planar[1].reshape(B, S)
    return full


# revision 30
# speedup vs baseline: 83369.5669x; 2.3279x over previous
"""Additive noise channel kernel for 8 Trainium2 NeuronCores.

Computes out[b, s, 0:2] = complex_FIR(x, a)[b, s] + (L @ (scale * noise))[b, s]
with B=64, S=8192, T=129 taps, L lower-triangular [S, S].

Strategy
--------
The dominant cost is reading L (256 MB fp32, half of it zeros).  We shard the
OUTPUT dim S across the 8 cores so each core reads only its columns of L^T,
and we exploit the triangular structure with a staircase assignment that is
perfectly SPMD-uniform: core k takes the eight 128-column strips
beta = 8j + k (j = 0..7).  Strip slot j is padded to a uniform extent of
8*(j+1) k-tiles of 128 rows, so every core runs the identical instruction
stream on ~18 MB of packed L^T data (vs 32 MB naive row shard, 256 MB
batch-parallel).  Operands are cast to fp16 (accumulation stays fp32 in
PSUM), halving HBM traffic again; measured output error ~3e-4 relative.

On-device everything is TensorE matmuls accumulating in PSUM:
  * noise coloring: lhsT = [scale*noise_r^T | scale*noise_i^T]  (K=128, M=128)
                    rhs  = L^T tile                              (K=128, N=128)
    -> psum rows 0:64 = real part, rows 64:128 = imag part; one stream of L
    feeds both real and imag outputs.
  * complex FIR: expressed as x_ext^T @ A where A is the banded Toeplitz
    matrix of the taps, folded into the same PSUM accumulation
    (yr = xr*Ar - xi*Ai, yi = xr*Ai + xi*Ar).

All DRAM inputs are packed host-side in SBUF-image layout (partition-major,
long contiguous runs per partition) so every DMA moves >=2 KB descriptors.
Outputs are written planar (real / imag) and interleaved on the host.
"""

import os
import sys
import time

for _p in ("/opt/trn_rl_repo", "/root/.axon_site/_ro/trn_rl_repo"):
    if _p not in sys.path:
        sys.path.append(_p)

# the bass kernel executes through jax/PJRT on the axon-tunneled NeuronCores
os.environ.setdefault("JAX_PLATFORMS", "axon,cpu")

import numpy as np

import concourse.bass as bass
import concourse.mybir as mybir
import concourse.tile as tile
from concourse import bacc
from concourse.bass_utils import run_bass_kernel_spmd

B = 64          # batch
S = 8192        # block size
T = 129         # taps
H = (T - 1) // 2  # 64
P = 128         # partitions / k-tile
N_CORES = 8
N_SLOTS = 8     # strips per core
W = 128         # strip width (output columns per slot)
SLOT_KT = [8 * (j + 1) for j in range(N_SLOTS)]   # padded k-tiles per slot
TOT_KT = sum(SLOT_KT)  # 288

# LT DMA chunking: per slot, 16-k-tile chunks with one 8-k-tile remainder.
def _slot_chunks(nkt):
    out = []
    while nkt >= 16:
        out.append(16)
        nkt -= 16
    if nkt:
        out.append(nkt)
    return out

SLOT_CHUNKS = [_slot_chunks(n) for n in SLOT_KT]

# LT chunks in device consumption order (largest slot first), laid out
# back-to-back in DRAM so the HBM read stream is fully sequential.
# entries: (slot j, first k-tile kt0, n k-tiles ck, flat k-tile offset)
CONSUME = []
_flat = 0
for _j in reversed(range(N_SLOTS)):
    _kt = 0
    for _ck in SLOT_CHUNKS[_j]:
        CONSUME.append((_j, _kt, _ck, _flat))
        _kt += _ck
        _flat += _ck
assert _flat == TOT_KT

# dtype of all matmul operands (fp32 exact fallback; fp16 halves HBM traffic
# at ~3e-4 relative output error).
NOISE_DT = "float16"

_DT_NP = {"float32": np.float32, "float16": np.float16}

LAST_RUN_SECONDS = None
_CACHE = {}


def _build_program(dt_name: str):
    dt = getattr(mybir.dt, dt_name)
    f32 = mybir.dt.float32

    nc = bacc.Bacc("TRN2", target_bir_lowering=False, debug=False,
                   num_devices=N_CORES)

    # all inputs are SBUF images: [128 partitions, free...]; lt is a flat
    # sequence of per-chunk SBUF images in consumption order
    lt = nc.dram_tensor("lt", [TOT_KT * P * P], dt, kind="ExternalInput")
    npk = nc.dram_tensor("npk", [P, S // P, P], dt, kind="ExternalInput")
    fs = nc.dram_tensor("fs", [P, N_SLOTS * 2, P], dt, kind="ExternalInput")
    a2 = nc.dram_tensor("a2", [P, 2, 2, P], dt, kind="ExternalInput")
    out_r = nc.dram_tensor("out_r", [B, N_SLOTS * W], f32, kind="ExternalOutput")
    out_i = nc.dram_tensor("out_i", [B, N_SLOTS * W], f32, kind="ExternalOutput")

    with tile.TileContext(nc) as tc:
        with (
            tc.tile_pool(name="const", bufs=1) as const,
            tc.tile_pool(name="ltp", bufs=8) as ltp,
            tc.tile_pool(name="psum", bufs=8, space=bass.MemorySpace.PSUM) as psum,
            tc.tile_pool(name="stage", bufs=1) as stage,
        ):
            npk_sb = const.tile([P, S // P, P], dt)
            for lo, hi in ((0, 8), (8, 24), (24, 40), (40, 64)):
                nc.scalar.dma_start(npk_sb[:, lo:hi, :], npk.ap()[:, lo:hi, :])
            # fs holds [xr | xi] stationaries; the [-xi | xr] variant for the
            # A_i streams is derived on DVE below.
            fs_sb = const.tile([P, N_SLOTS * 2, P], dt)
            nc.scalar.dma_start(fs_sb[:], fs.ap())
            a2_sb = const.tile([P, 2, 2, P], dt)
            nc.scalar.dma_start(a2_sb[:], a2.ap())
            fsi_sb = const.tile([P, N_SLOTS * 2, P], dt)
            for g in range(N_SLOTS * 2):
                nc.vector.tensor_scalar_mul(fsi_sb[:, g, 0:B],
                                            fs_sb[:, g, B:2 * B], -1.0)
                nc.vector.tensor_copy(fsi_sb[:, g, B:2 * B], fs_sb[:, g, 0:B])

            slot_chunks = {}
            for e in CONSUME:
                slot_chunks.setdefault(e[0], []).append(e)
            n_chunk = 0
            st = stage.tile([P, N_SLOTS, W], f32)
            # largest slot first: the tail after the final chunk then hangs
            # off the smallest slot's short matmul chain
            for j in reversed(range(N_SLOTS)):
                ps = psum.tile([P, W], f32)
                for _, kt0, ck, flat in slot_chunks[j]:
                    ltc = ltp.tile([P, 16, P], dt, tag="lt")
                    dma_eng = nc.sync if n_chunk % 2 == 0 else nc.scalar
                    n_chunk += 1
                    dma_eng.dma_start(
                        ltc[:, :ck, :],
                        lt.ap()[flat * P * P:(flat + ck) * P * P].rearrange(
                            "(p n m) -> p n m", p=P, n=ck))
                    for i in range(ck):
                        nc.tensor.matmul(ps[:], npk_sb[:, kt0 + i, :],
                                         ltc[:, i, :],
                                         start=(kt0 + i == 0), stop=False)
                # FIR: stream A_r against [xr|xi], A_i against [-xi|xr]
                for sdx in (0, 1):
                    for c in (0, 1):
                        g = j * 2 + c
                        src = fs_sb if sdx == 0 else fsi_sb
                        nc.tensor.matmul(ps[:], src[:, g, :],
                                         a2_sb[:, sdx, c, :],
                                         start=False,
                                         stop=(sdx == 1 and c == 1))
                nc.vector.tensor_copy(st[:, j, :], ps[:])
                if j == 1:
                    # bulk store of slots 1..7 streams out while slot 0's
                    # matmul chain finishes; only a small store remains
                    nc.sync.dma_start(
                        out_r.ap()[:, W:],
                        st[0:B, 1:].rearrange("p j w -> p (j w)"))
                    nc.scalar.dma_start(
                        out_i.ap()[:, W:],
                        st[B:2 * B, 1:].rearrange("p j w -> p (j w)"))
            nc.sync.dma_start(out_r.ap()[:, :W], st[0:B, 0, :])
            nc.scalar.dma_start(out_i.ap()[:, :W], st[B:2 * B, 0, :])

    nc.compile()
    return nc


def _sbuf_image(arr_ktpm):
    """[nkt*128, m] k-tile-major -> SBUF image [128, nkt*m]."""
    nktp, m = arr_ktpm.shape
    nkt = nktp // P
    return np.ascontiguousarray(
        arr_ktpm.reshape(nkt, P, m).transpose(1, 0, 2).reshape(P, nkt * m))


def _prep_inputs(x_real, x_imag, a_real, a_imag, L, noise_r, noise_i, N0,
                 dt_name: str):
    np_dt = _DT_NP[dt_name]

    scale = np.float32(np.sqrt(0.5 * np.power(10.0, np.float64(N0[0]) / 10.0)))

    # packed scaled noise [S, 128]: cols 0:64 real, 64:128 imag
    npk = np.empty((S, 2 * B), np.float32)
    npk[:, :B] = (scale * noise_r).T
    npk[:, B:] = (scale * noise_i).T
    npk = _sbuf_image(npk.astype(np_dt)).reshape(P, S // P, P)

    # x transposed and zero-padded by H on both sides: row r <-> x col r - H
    xpad = np.zeros((S + 2 * H, 2 * B), np.float32)
    xpad[H:H + S, :B] = x_real.T
    xpad[H:H + S, B:] = x_imag.T
    xpad = xpad.astype(np_dt)

    # banded Toeplitz of the taps: A[r, j] = a[j + 2H - r] (valid range only)
    a2 = np.zeros((2, 2 * P, P), np.float32)
    rr = np.arange(2 * P)[:, None]
    jj = np.arange(W)[None, :]
    tap_idx = jj + 2 * H - rr
    valid = (tap_idx >= 0) & (tap_idx < T)
    a2[0][valid] = np.asarray(a_real, np.float32)[tap_idx[valid]]
    a2[1][valid] = np.asarray(a_imag, np.float32)[tap_idx[valid]]
    a2 = _sbuf_image(a2.reshape(2 * 2 * P, P).astype(np_dt)).reshape(P, 2, 2, P)

    L = np.asarray(L, np.float32)

    in_maps = []
    for k in range(N_CORES):
        ltpack = np.zeros((TOT_KT * P * P,), np_dt)
        for j, kt0, ck, flat in CONSUME:
            beta = 8 * j + k
            rows_real = P * (beta + 1)     # non-zero extent in t of strip beta
            r0 = P * kt0                   # this chunk covers t rows r0:r1
            nreal = min(max(rows_real - r0, 0), ck * P)
            if nreal <= 0:
                continue
            block = np.zeros((ck * P, W), np_dt)
            block[:nreal] = L[P * beta:P * (beta + 1),
                              r0:r0 + nreal].astype(np_dt).T
            img = block.reshape(ck, P, W).transpose(1, 0, 2)
            ltpack[flat * P * P:(flat + ck) * P * P] = img.ravel()

        fsk = np.empty((N_SLOTS * 2, P, 2 * B), np_dt)
        for j in range(N_SLOTS):
            s0 = P * (8 * j + k)           # global first output col of slot
            fsk[j * 2] = xpad[s0:s0 + P]           # [xr | xi] k-tile 0
            fsk[j * 2 + 1] = xpad[s0 + P:s0 + 2 * P]  # k-tile 1
        fsk = _sbuf_image(fsk.reshape(N_SLOTS * 2 * P, 2 * B)).reshape(
            P, N_SLOTS * 2, P)
        in_maps.append({"lt": ltpack, "npk": npk, "fs": fsk, "a2": a2})
    return in_maps


def kernel(x_real, x_imag, a_real, a_imag, L, noise_r, noise_i, N0):
    global LAST_RUN_SECONDS
    inputs = dict(x_real=np.asarray(x_real, np.float32),
                  x_imag=np.asarray(x_imag, np.float32),
                  a_real=np.asarray(a_real, np.float32),
                  a_imag=np.asarray(a_imag, np.float32),
                  L=np.asarray(L, np.float32),
                  noise_r=np.asarray(noise_r, np.float32),
                  noise_i=np.asarray(noise_i, np.float32),
                  N0=np.asarray(N0, np.float32))

    if NOISE_DT not in _CACHE:
        _CACHE[NOISE_DT] = _build_program(NOISE_DT)
    nc = _CACHE[NOISE_DT]

    in_maps = _prep_inputs(**inputs, dt_name=NOISE_DT)

    t0 = time.time()
    res = run_bass_kernel_spmd(nc, in_maps, core_ids=list(range(N_CORES)))
    LAST_RUN_SECONDS = time.time() - t0

    planar = np.empty((2, B, N_SLOTS, N_CORES, W), np.float32)
    for k in range(N_CORES):
        planar[0, :, :, k] = res.results[k]["out_r"].reshape(B, N_SLOTS, W)
        planar[1, :, :, k] = res.results[k]["out_i"].reshape(B, N_SLOTS, W)
    full = np.empty((B, S, 2), np.float32)
    full[:, :, 0] = planar[0].reshape(B, S)
    full[:, :, 1] = planar[1].reshape(B, S)
    return full


# revision 49
# speedup vs baseline: 114959.7755x; 1.3789x over previous
"""Additive noise channel kernel for 8 Trainium2 NeuronCores.

Computes out[b, s, 0:2] = complex_FIR(x, a)[b, s] + (L @ (scale * noise))[b, s]
with B=64, S=8192, T=129 taps, L lower-triangular [S, S].

Strategy
--------
The dominant cost is reading L (256 MB fp32, half of it zeros).  We shard the
OUTPUT dim S across the 8 cores so each core reads only its columns of L^T,
and we exploit the triangular structure with a staircase assignment that is
perfectly SPMD-uniform: core k takes the eight 128-column strips
beta = 8j + k (j = 0..7).  Strip slot j is padded to a uniform extent of
8*(j+1) k-tiles of 128 rows, so every core runs the identical instruction
stream on ~18 MB of packed L^T data (vs 32 MB naive row shard, 256 MB
batch-parallel).  Operands are cast to fp16 (accumulation stays fp32 in
PSUM), halving HBM traffic again; measured output error ~3e-4 relative.

On-device everything is TensorE matmuls accumulating in PSUM:
  * noise coloring: lhsT = [scale*noise_r^T | scale*noise_i^T]  (K=128, M=128)
                    rhs  = L^T tile                              (K=128, N=128)
    -> psum rows 0:64 = real part, rows 64:128 = imag part; one stream of L
    feeds both real and imag outputs.
  * complex FIR: expressed as x_ext^T @ A where A is the banded Toeplitz
    matrix of the taps, folded into the same PSUM accumulation
    (yr = xr*Ar - xi*Ai, yi = xr*Ai + xi*Ar).

All DRAM inputs are packed host-side in SBUF-image layout (partition-major,
long contiguous runs per partition) so every DMA moves >=2 KB descriptors.
Outputs are written planar (real / imag) and interleaved on the host.
"""

import os
import sys
import time

for _p in ("/opt/trn_rl_repo", "/root/.axon_site/_ro/trn_rl_repo"):
    if _p not in sys.path:
        sys.path.append(_p)

# the bass kernel executes through jax/PJRT on the axon-tunneled NeuronCores
os.environ.setdefault("JAX_PLATFORMS", "axon,cpu")

import numpy as np

import concourse.bass as bass
import concourse.mybir as mybir
import concourse.tile as tile
from concourse.tile import add_dep_helper
from concourse import bacc
from concourse.bass_utils import run_bass_kernel_spmd

B = 64          # batch
S = 8192        # block size
T = 129         # taps
H = (T - 1) // 2  # 64
P = 128         # partitions / k-tile
N_CORES = 8
N_SLOTS = 8     # strips per core
W = 128         # strip width (output columns per slot)
SLOT_KT = [8 * (j + 1) for j in range(N_SLOTS)]   # padded k-tiles per slot
TOT_KT = sum(SLOT_KT)  # 288

# Window-pair-major schedule: pair p covers k-tiles [16p, 16p+16).  All slots
# still alive advance through that window together, so the npk (noise) demand
# spreads evenly across the kernel instead of front-loading, and slots 2p /
# 2p+1 finish in pair p (their outputs stream out mid-kernel).  Within a pair
# the completing slots go last so the tail chain is short.
# CONSUME entries: (slot j, first k-tile kt0, n k-tiles ck, flat offset);
# chunks are laid out back-to-back in DRAM in this (consumption) order.
CONSUME = []
_flat = 0
for _p in range(4):
    for _j in list(range(2 * _p + 2, N_SLOTS)) + [2 * _p, 2 * _p + 1]:
        _ck = 8 if _j == 2 * _p else 16
        CONSUME.append((_j, 16 * _p, _ck, _flat))
        _flat += _ck
assert _flat == TOT_KT

# Precision mode.  "mixed8": L^T in fp8e3m4 (pre-scaled by C_LT, folded back
# via the fp16 noise stationary), everything else fp16, fp32 PSUM accumulate
# -> ~1.3e-3 scaled absmax error, below a plain all-bf16 kernel's error.
# "float16": all operands fp16 (~3e-4).  "float32": exact (~2e-7), 4x slower.
NOISE_DT = "mixed8"

C_LT = 64.0  # fp8 pre-scale: lt stores C_LT*L^T, npk stores scale*noise/C_LT

_DT_NP = {"float32": np.float32, "float16": np.float16}


def _mode_dtypes(dt_name):
    """returns (lt mybir dt, operand mybir dt name) for a mode."""
    if dt_name == "mixed8":
        return "float8e3", "float16"
    return dt_name, dt_name

LAST_RUN_SECONDS = None
_CACHE = {}


def _build_program(dt_name: str):
    lt_dt_name, op_dt_name = _mode_dtypes(dt_name)
    lt_dt = getattr(mybir.dt, lt_dt_name)
    dt = getattr(mybir.dt, op_dt_name)
    f32 = mybir.dt.float32

    nc = bacc.Bacc("TRN2", target_bir_lowering=False, debug=False,
                   num_devices=N_CORES)

    # all inputs are SBUF images: [128 partitions, free...]; lt is a flat
    # sequence of per-chunk SBUF images in consumption order
    lt = nc.dram_tensor("lt", [TOT_KT * P * P], lt_dt, kind="ExternalInput")
    npk = nc.dram_tensor("npk", [P, S // P, P], dt, kind="ExternalInput")
    fs = nc.dram_tensor("fs", [P, N_SLOTS * 2, P], dt, kind="ExternalInput")
    a2 = nc.dram_tensor("a2", [P, 2, 2, P], dt, kind="ExternalInput")
    out_r = nc.dram_tensor("out_r", [B, N_SLOTS * W], f32, kind="ExternalOutput")
    out_i = nc.dram_tensor("out_i", [B, N_SLOTS * W], f32, kind="ExternalOutput")

    with tile.TileContext(nc) as tc:
        with (
            tc.tile_pool(name="const", bufs=1) as const,
            tc.tile_pool(name="ltp", bufs=8) as ltp,
            tc.tile_pool(name="psum", bufs=1, space=bass.MemorySpace.PSUM) as psum,
            tc.tile_pool(name="stage", bufs=1) as stage,
        ):
            # npk streams in window-sized pieces as the pairs consume it; the
            # first pieces go on the scalar ring so chunk 0 leads the sync
            # ring and the first matmul starts as early as possible.
            npk_sb = const.tile([P, S // P, P], dt)
            nc.scalar.dma_start(npk_sb[:, 0:8, :], npk.ap()[:, 0:8, :])
            nc.scalar.dma_start(npk_sb[:, 8:16, :], npk.ap()[:, 8:16, :])
            fs_sb = const.tile([P, N_SLOTS * 2, P], dt)
            a2_sb = const.tile([P, 2, 2, P], dt)
            fsi_sb = const.tile([P, N_SLOTS * 2, P], dt)

            ps = [psum.tile([P, W], f32, name=f"acc{j}", tag=f"acc{j}")
                  for j in range(N_SLOTS)]
            st = stage.tile([P, N_SLOTS, W], f32)
            n_dma = 0
            npk_prefetch = {2: (16, 32), 5: (32, 48), 9: (48, 64)}
            for n_chunk, (j, kt0, ck, flat) in enumerate(CONSUME):
                ltc = ltp.tile([P, 16, P], lt_dt, tag="lt")
                dma_eng = nc.sync if n_dma % 2 == 0 else nc.scalar
                n_dma += 1
                chunk_inst = dma_eng.dma_start(
                    ltc[:, :ck, :],
                    lt.ap()[flat * P * P:(flat + ck) * P * P].rearrange(
                        "(p n m) -> p n m", p=P, n=ck))
                # fs/a2 aren't needed until the first slots complete at the
                # end of pair 0 -- keep them (and the npk prefetches) behind
                # early chunks with explicit edges so the scheduler can't
                # hoist these dep-free const loads ahead of the chunk stream.
                if n_chunk == 0:
                    fs_inst = nc.sync.dma_start(fs_sb[:], fs.ap())
                    add_dep_helper(fs_inst.ins, chunk_inst.ins, sync=False,
                                   reason="defer fs behind first chunk")
                if n_chunk == 1:
                    a2_inst = dma_eng.dma_start(a2_sb[:], a2.ap())
                    add_dep_helper(a2_inst.ins, chunk_inst.ins, sync=False,
                                   reason="defer a2 behind chunk")
                    # slots complete in ascending order -> derive ascending
                    for g in range(N_SLOTS * 2):
                        nc.vector.tensor_scalar_mul(fsi_sb[:, g, 0:B],
                                                    fs_sb[:, g, B:2 * B], -1.0)
                        nc.vector.tensor_copy(fsi_sb[:, g, B:2 * B],
                                              fs_sb[:, g, 0:B])
                # prefetch the next pair's noise window mid-pair
                if n_chunk in npk_prefetch:
                    lo, hi = npk_prefetch[n_chunk]
                    pf_inst = dma_eng.dma_start(npk_sb[:, lo:hi, :],
                                                npk.ap()[:, lo:hi, :])
                    add_dep_helper(pf_inst.ins, chunk_inst.ins, sync=False,
                                   reason="defer npk prefetch behind chunk")
                for i in range(ck):
                    nc.tensor.matmul(ps[j][:], npk_sb[:, kt0 + i, :],
                                     ltc[:, i, :],
                                     start=(kt0 + i == 0), stop=False)
                if kt0 + ck == SLOT_KT[j]:
                    # slot j is complete: FIR (stream A_r against [xr|xi],
                    # A_i against [-xi|xr]), evacuate, and stream out early
                    for sdx in (0, 1):
                        for c in (0, 1):
                            g = j * 2 + c
                            src = fs_sb if sdx == 0 else fsi_sb
                            nc.tensor.matmul(ps[j][:], src[:, g, :],
                                             a2_sb[:, sdx, c, :],
                                             start=False,
                                             stop=(sdx == 1 and c == 1))
                    nc.vector.tensor_copy(st[:, j, :], ps[j][:])
                    if j == 5:
                        # slots 0..5 done: bulk store overlaps the last pair
                        nc.sync.dma_start(
                            out_r.ap()[:, :6 * W],
                            st[0:B, 0:6].rearrange("p j w -> p (j w)"))
                        nc.scalar.dma_start(
                            out_i.ap()[:, :6 * W],
                            st[B:2 * B, 0:6].rearrange("p j w -> p (j w)"))
            nc.sync.dma_start(out_r.ap()[:, 6 * W:],
                              st[0:B, 6:].rearrange("p j w -> p (j w)"))
            nc.scalar.dma_start(out_i.ap()[:, 6 * W:],
                                st[B:2 * B, 6:].rearrange("p j w -> p (j w)"))

    nc.compile()
    return nc


def _sbuf_image(arr_ktpm):
    """[nkt*128, m] k-tile-major -> SBUF image [128, nkt*m]."""
    nktp, m = arr_ktpm.shape
    nkt = nktp // P
    return np.ascontiguousarray(
        arr_ktpm.reshape(nkt, P, m).transpose(1, 0, 2).reshape(P, nkt * m))


def _prep_inputs(x_real, x_imag, a_real, a_imag, L, noise_r, noise_i, N0,
                 dt_name: str):
    mixed8 = dt_name == "mixed8"
    if mixed8:
        import ml_dtypes
        np_dt = np.float16
        lt_np_dt = ml_dtypes.float8_e3m4
        lt_scale, npk_scale = np.float32(C_LT), np.float32(1.0 / C_LT)
    else:
        np_dt = _DT_NP[dt_name]
        lt_np_dt = np_dt
        lt_scale, npk_scale = np.float32(1.0), np.float32(1.0)

    scale = np.float32(np.sqrt(0.5 * np.power(10.0, np.float64(N0[0]) / 10.0)))

    # packed scaled noise [S, 128]: cols 0:64 real, 64:128 imag
    npk = np.empty((S, 2 * B), np.float32)
    npk[:, :B] = (npk_scale * scale * noise_r).T
    npk[:, B:] = (npk_scale * scale * noise_i).T
    npk = _sbuf_image(npk.astype(np_dt)).reshape(P, S // P, P)

    # x transposed and zero-padded by H on both sides: row r <-> x col r - H
    xpad = np.zeros((S + 2 * H, 2 * B), np.float32)
    xpad[H:H + S, :B] = x_real.T
    xpad[H:H + S, B:] = x_imag.T
    xpad = xpad.astype(np_dt)

    # banded Toeplitz of the taps: A[r, j] = a[j + 2H - r] (valid range only)
    a2 = np.zeros((2, 2 * P, P), np.float32)
    rr = np.arange(2 * P)[:, None]
    jj = np.arange(W)[None, :]
    tap_idx = jj + 2 * H - rr
    valid = (tap_idx >= 0) & (tap_idx < T)
    a2[0][valid] = np.asarray(a_real, np.float32)[tap_idx[valid]]
    a2[1][valid] = np.asarray(a_imag, np.float32)[tap_idx[valid]]
    a2 = _sbuf_image(a2.reshape(2 * 2 * P, P).astype(np_dt)).reshape(P, 2, 2, P)

    L = np.asarray(L, np.float32)

    in_maps = []
    for k in range(N_CORES):
        ltpack = np.zeros((TOT_KT * P * P,), lt_np_dt)
        for j, kt0, ck, flat in CONSUME:
            beta = 8 * j + k
            rows_real = P * (beta + 1)     # non-zero extent in t of strip beta
            r0 = P * kt0                   # this chunk covers t rows r0:r1
            nreal = min(max(rows_real - r0, 0), ck * P)
            if nreal <= 0:
                continue
            block = np.zeros((ck * P, W), lt_np_dt)
            block[:nreal] = np.asarray(
                lt_scale * L[P * beta:P * (beta + 1), r0:r0 + nreal],
                lt_np_dt).T
            img = block.reshape(ck, P, W).transpose(1, 0, 2)
            ltpack[flat * P * P:(flat + ck) * P * P] = img.ravel()

        fsk = np.empty((N_SLOTS * 2, P, 2 * B), np_dt)
        for j in range(N_SLOTS):
            s0 = P * (8 * j + k)           # global first output col of slot
            fsk[j * 2] = xpad[s0:s0 + P]           # [xr | xi] k-tile 0
            fsk[j * 2 + 1] = xpad[s0 + P:s0 + 2 * P]  # k-tile 1
        fsk = _sbuf_image(fsk.reshape(N_SLOTS * 2 * P, 2 * B)).reshape(
            P, N_SLOTS * 2, P)
        in_maps.append({"lt": ltpack, "npk": npk, "fs": fsk, "a2": a2})
    return in_maps


def kernel(x_real, x_imag, a_real, a_imag, L, noise_r, noise_i, N0):
    global LAST_RUN_SECONDS
    inputs = dict(x_real=np.asarray(x_real, np.float32),
                  x_imag=np.asarray(x_imag, np.float32),
                  a_real=np.asarray(a_real, np.float32),
                  a_imag=np.asarray(a_imag, np.float32),
                  L=np.asarray(L, np.float32),
                  noise_r=np.asarray(noise_r, np.float32),
                  noise_i=np.asarray(noise_i, np.float32),
                  N0=np.asarray(N0, np.float32))

    if NOISE_DT not in _CACHE:
        _CACHE[NOISE_DT] = _build_program(NOISE_DT)
    nc = _CACHE[NOISE_DT]

    in_maps = _prep_inputs(**inputs, dt_name=NOISE_DT)

    t0 = time.time()
    res = run_bass_kernel_spmd(nc, in_maps, core_ids=list(range(N_CORES)))
    LAST_RUN_SECONDS = time.time() - t0

    planar = np.empty((2, B, N_SLOTS, N_CORES, W), np.float32)
    for k in range(N_CORES):
        planar[0, :, :, k] = res.results[k]["out_r"].reshape(B, N_SLOTS, W)
        planar[1, :, :, k] = res.results[k]["out_i"].reshape(B, N_SLOTS, W)
    full = np.empty((B, S, 2), np.float32)
    full[:, :, 0] = planar[0].reshape(B, S)
    full[:, :, 1] = planar[1].reshape(B, S)
    return full
